# revision 1
# baseline (speedup 1.0000x reference)
"""Trainium2 Bass kernel for single-head causal attention (B=4, T=2048, C=2048).

Sharding: 8 cores = 4 batches x 2 t-interleave. Core (b, h) owns the 256-row
blocks {h, 2+h, 4+h, 6+h} of batch b (interleaved for causal load balance).
The two cores of a batch each compute HALF of K.T and V and exchange via a
2-core AllGather (overlapped with the Q projection). Attention runs in the
"transposed domain" (scores.T = [s, t]) so every matmul consumes naturally
laid-out operands: exp(scale*s + additive mask) without normalization, softmax
denominators via ones-matmul partition reduction, folded in as a per-partition
scale on the final-projection output, which lands in natural [t, e] layout.
Host pre-transposes x / weights (part of sharding prep) and gathers per-core
outputs. All matmuls run in fp32r (full PE rate, ~tf32 precision).
"""
import sys

sys.path.insert(0, "/opt/trn_rl_repo")
import numpy as np

_CACHE = {}

B = 4
T_FULL = 2048
C_FULL = 2048
NEG = -1e30


def _build(T_, C_, reps=1):
    import concourse.bacc as bacc
    import concourse.mybir as mybir
    import concourse.tile as tile

    F32 = mybir.dt.float32
    F32R = mybir.dt.float32r
    AF = mybir.ActivationFunctionType
    SCALE = 1.0 / float(np.sqrt(C_FULL))

    CC = C_ // 128      # contraction 128-chunks (also d-chunks)
    NE = C_ // 512      # e-512 chunks for the final projection
    NBO = T_ // 512     # owned 256-blocks per core (j range)
    TOWN = NBO * 256    # owned rows per core
    SK = T_ // 512      # s-512 chunks (KT scratch blocks)
    NS = T_ // 128      # s-128 chunks
    NQ = TOWN // 512    # q 512-col chunks

    nc = bacc.Bacc("TRN2", target_bir_lowering=False, debug=False, num_devices=8)
    xTh_d = nc.declare_dram_parameter("xTh", [C_, T_ // 2], F32R, isOutput=False)
    xTq_d = nc.declare_dram_parameter("xTq", [C_, TOWN], F32R, isOutput=False)
    WqT_d = nc.declare_dram_parameter("WqT", [C_, C_], F32R, isOutput=False)
    WkT_d = nc.declare_dram_parameter("WkT", [C_, C_], F32R, isOutput=False)
    WvT_d = nc.declare_dram_parameter("WvT", [C_, C_], F32R, isOutput=False)
    WoT_d = nc.declare_dram_parameter("WoT", [C_, C_], F32R, isOutput=False)
    mb_d = nc.declare_dram_parameter("mb", [NBO, 4, 128, 256], F32, isOutput=False)
    ones_d = nc.declare_dram_parameter("ones", [128, 2], F32R, isOutput=False)
    y_d = nc.declare_dram_parameter("y", [TOWN, C_], F32, isOutput=True)

    with tile.TileContext(nc) as tc:
        with tc.tile_pool(name="dram", bufs=1, space="DRAM") as dram:
            KT_s = dram.tile([SK, C_, 512], F32R, tag="kts")
            V_s = dram.tile([T_, C_], F32R, tag="vs")
            KT_h = dram.tile([SK // 2, C_, 512], F32R, tag="kth")
            V_h = dram.tile([T_ // 2, C_], F32R, tag="vh")
            QT_s = dram.tile([C_, TOWN], F32R, tag="qts")
            OT_s = dram.tile([C_, TOWN], F32R, tag="ots")

            with tc.tile_pool(name="stage", bufs=4) as stage:
                for _rep in range(reps):
                    # ======== P1a: K.T = WkT.T @ xT  -> KT_s [d, s] ========
                    with tc.tile_pool(name="xt", bufs=1) as pool_xt:
                        xt = pool_xt.tile([128, CC, T_ // 2], F32R, tag="xt")
                        for c in range(CC):
                            nc.sync.dma_start(xt[:, c, :], xTh_d[128 * c:128 * c + 128, :])
                        with (
                            tc.tile_pool(name="wk", bufs=2) as pool_w,
                            tc.tile_pool(name="psk", bufs=8, space="PSUM") as psk,
                        ):
                            for d in range(CC):
                                wk = pool_w.tile([128, CC, 128], F32R, tag="wk")
                                nc.sync.dma_start(
                                    wk[:],
                                    WkT_d[:, 128 * d:128 * d + 128].rearrange(
                                        "(cc p) d -> p cc d", p=128
                                    ),
                                )
                                kps = [psk.tile([128, 512], F32, tag="kps", name=f"kps{d}_{ss}") for ss in range(SK // 2)]
                                for c in range(CC):
                                    for ss in range(SK // 2):
                                        nc.tensor.matmul(
                                            kps[ss][:],
                                            wk[:, c, :],
                                            xt[:, c, 512 * ss:512 * ss + 512],
                                            start=(c == 0),
                                            stop=(c == CC - 1),
                                        )
                                for ss in range(SK // 2):
                                    st = stage.tile([128, 512], F32R, tag="st512")
                                    nc.vector.tensor_copy(st[:], kps[ss][:])
                                    nc.sync.dma_start(
                                        KT_h[ss, 128 * d:128 * d + 128, :], st[:]
                                    )
                        # ======== P1b: V = xT.T @ WvT -> V_s [s, d] ========
                        with (
                            tc.tile_pool(name="wv", bufs=2) as pool_wv,
                            tc.tile_pool(name="psv", bufs=4, space="PSUM") as psv,
                        ):
                            for dd in range(C_ // 512):
                                wv = pool_wv.tile([128, CC, 512], F32R, tag="wv")
                                nc.sync.dma_start(
                                    wv[:],
                                    WvT_d[:, 512 * dd:512 * dd + 512].rearrange(
                                        "(cc p) d -> p cc d", p=128
                                    ),
                                )
                                for s in range(NS // 2):
                                    vps = psv.tile([128, 512], F32, tag="vps")
                                    for c in range(CC):
                                        nc.tensor.matmul(
                                            vps[:],
                                            xt[:, c, 128 * s:128 * s + 128],
                                            wv[:, c, :],
                                            start=(c == 0),
                                            stop=(c == CC - 1),
                                        )
                                    st = stage.tile([128, 512], F32R, tag="st512")
                                    nc.vector.tensor_copy(st[:], vps[:])
                                    nc.sync.dma_start(
                                        V_h[128 * s:128 * s + 128, 512 * dd:512 * dd + 512],
                                        st[:],
                                    )
                    # ======== exchange K/V halves within batch pairs ========
                    groups = [[0, 1], [2, 3], [4, 5], [6, 7]]
                    nc.gpsimd.collective_compute(
                        "AllGather",
                        mybir.AluOpType.bypass,
                        replica_groups=groups,
                        ins=[KT_h[:]],
                        outs=[KT_s[:]],
                    )
                    nc.gpsimd.collective_compute(
                        "AllGather",
                        mybir.AluOpType.bypass,
                        replica_groups=groups,
                        ins=[V_h[:]],
                        outs=[V_s[:]],
                    )
                    # ======== P1c: Q.T = WqT.T @ xTq -> QT_s [d, t_packed] ========
                    with (
                        tc.tile_pool(name="xtq", bufs=1) as pool_xtq,
                        tc.tile_pool(name="wq", bufs=2) as pool_wq,
                        tc.tile_pool(name="psq", bufs=4, space="PSUM") as psq,
                    ):
                        xtq = pool_xtq.tile([128, CC, TOWN], F32R, tag="xtq")
                        for c in range(CC):
                            nc.sync.dma_start(xtq[:, c, :], xTq_d[128 * c:128 * c + 128, :])
                        for d in range(CC):
                            wq = pool_wq.tile([128, CC, 128], F32R, tag="wq")
                            nc.sync.dma_start(
                                wq[:],
                                WqT_d[:, 128 * d:128 * d + 128].rearrange(
                                    "(cc p) d -> p cc d", p=128
                                ),
                            )
                            for tt in range(NQ):
                                qps = psq.tile([128, 512], F32, tag="qps")
                                for c in range(CC):
                                    nc.tensor.matmul(
                                        qps[:],
                                        wq[:, c, :],
                                        xtq[:, c, 512 * tt:512 * tt + 512],
                                        start=(c == 0),
                                        stop=(c == CC - 1),
                                    )
                                st = stage.tile([128, 512], F32R, tag="st512")
                                nc.vector.tensor_copy(st[:], qps[:])
                                nc.sync.dma_start(
                                    QT_s[128 * d:128 * d + 128, 512 * tt:512 * tt + 512],
                                    st[:],
                                )

                    # ======== P2: attention per owned block j ========
                    pool_cst_cm = tc.tile_pool(name="cst", bufs=1)
                    pool_cst = pool_cst_cm.__enter__()
                    onest = pool_cst.tile([128, 2], F32R, tag="ones")
                    nc.sync.dma_start(onest[:], ones_d[:])
                    recipt = pool_cst.tile([128, 2 * NBO], F32, tag="recip")
                    with (
                        tc.tile_pool(name="mbp", bufs=1) as pool_mb,
                        tc.tile_pool(name="qt", bufs=2) as pool_qt,
                        tc.tile_pool(name="kt", bufs=2) as pool_kt,
                        tc.tile_pool(name="vp", bufs=3) as pool_vp,
                        tc.tile_pool(name="attn", bufs=8 * NBO - 4) as pool_attn,
                    ):
                        mbt = pool_mb.tile([128, NBO, 4, 256], F32, tag="mb")
                        nc.sync.dma_start(mbt[:], mb_d[:].rearrange("nb k p n -> p nb k n"))

                        # owned blocks processed in PAIRS so one K-chunk weight
                        # load serves two scores matmuls (fp32r self-loads per
                        # MM; rhs multiplicity is the only amortization)
                        for grp in range(NBO // 2):
                            js = [2 * grp, 2 * grp + 1]
                            qtps = {}
                            attn = {}
                            for j in js:
                                qtp = pool_qt.tile(
                                    [128, CC, 256], F32R, tag="qt", name=f"qtp{j}"
                                )
                                nc.sync.dma_start(
                                    qtp[:],
                                    QT_s[:, 256 * j:256 * j + 256].rearrange(
                                        "(cc p) t -> p cc t", p=128
                                    ),
                                )
                                qtps[j] = qtp
                                attn[j] = [
                                    pool_attn.tile(
                                        [128, 256], F32R, tag="attn", name=f"attn{j}_{k}"
                                    )
                                    for k in range(4 * j + 4)
                                ]
                            with tc.tile_pool(name="pssc", bufs=4, space="PSUM") as pssc:
                                for kk in range(2 * grp + 2):
                                    ktp = pool_kt.tile([128, CC, 512], F32R, tag="kt")
                                    nc.sync.dma_start(
                                        ktp[:],
                                        KT_s[kk].rearrange("(cc p) s -> p cc s", p=128),
                                    )
                                    for kl in range(4):
                                        k = 4 * kk + kl
                                        jlist = [j for j in js if 4 * j + 3 >= k]
                                        sps = {
                                            j: pssc.tile(
                                                [128, 256], F32, tag="sps",
                                                name=f"sps{grp}_{k}_{j}",
                                            )
                                            for j in jlist
                                        }
                                        for d in range(CC):
                                            for j in jlist:
                                                nc.tensor.matmul(
                                                    sps[j][:],
                                                    ktp[:, d, 128 * kl:128 * kl + 128],
                                                    qtps[j][:, d, :],
                                                    start=(d == 0),
                                                    stop=(d == CC - 1),
                                                )
                                        for j in jlist:
                                            if k >= 4 * j:
                                                nc.vector.tensor_add(
                                                    sps[j][:], sps[j][:],
                                                    mbt[:, j, k - 4 * j, :],
                                                )
                                            nc.scalar.activation(
                                                attn[j][k][:], sps[j][:], AF.Exp,
                                                scale=SCALE,
                                            )
                            with tc.tile_pool(name="psr", bufs=2, space="PSUM") as psr:
                                for j in js:
                                    for sub in range(2):
                                        rps = psr.tile([128, 2], F32, tag="rps")
                                        for k in range(4 * j + 4):
                                            nc.tensor.matmul(
                                                rps[:],
                                                attn[j][k][:, 128 * sub:128 * sub + 128],
                                                onest[:],
                                                start=(k == 0),
                                                stop=(k == 4 * j + 3),
                                            )
                                        nc.vector.reciprocal(
                                            recipt[:, 2 * j + sub:2 * j + sub + 1],
                                            rps[:, 0:1],
                                        )
                            for j in js:
                                n_k = 4 * j + 4
                                with tc.tile_pool(
                                    name="psav", bufs=CC // 2, space="PSUM"
                                ) as psav:
                                    avs = [
                                        psav.tile(
                                            [128, 512], F32, tag="av", name=f"av{j}_{dp}"
                                        )
                                        for dp in range(CC // 2)
                                    ]
                                    for dp in range(CC // 2):
                                        nc.vector.memset(avs[dp][:], 0.0)
                                    for k in range(n_k):
                                        vp = pool_vp.tile([128, C_], F32R, tag="vp")
                                        nc.sync.dma_start(
                                            vp[:], V_s[128 * k:128 * k + 128, :]
                                        )
                                        for d in range(CC):
                                            nc.tensor.matmul(
                                                avs[d // 2][:, 256 * (d % 2):256 * (d % 2) + 256],
                                                vp[:, 128 * d:128 * d + 128],
                                                attn[j][k][:],
                                                start=False,
                                                stop=(k == n_k - 1),
                                                skip_group_check=True,
                                            )
                                    for d in range(CC):
                                        st = stage.tile([128, 256], F32R, tag="st256")
                                        nc.vector.tensor_copy(
                                            st[:],
                                            avs[d // 2][:, 256 * (d % 2):256 * (d % 2) + 256],
                                        )
                                        nc.sync.dma_start(
                                            OT_s[128 * d:128 * d + 128, 256 * j:256 * j + 256],
                                            st[:],
                                        )

                    # ======== P3: y = (OT.T @ WoT) * recip ========
                    with (
                        tc.tile_pool(name="wo", bufs=2) as pool_wo,
                        tc.tile_pool(name="ot", bufs=2 * NBO) as pool_ot,
                        tc.tile_pool(name="psf", bufs=4, space="PSUM") as psf,
                    ):
                        otps = []
                        for tsub in range(2 * NBO):
                            otp = pool_ot.tile([128, CC, 128], F32R, tag="ot", name=f"otp{tsub}")
                            nc.sync.dma_start(
                                otp[:],
                                OT_s[:, 128 * tsub:128 * tsub + 128].rearrange(
                                    "(cc p) t -> p cc t", p=128
                                ),
                            )
                            otps.append(otp)
                        for e in range(NE):
                            wo = pool_wo.tile([128, CC, 512], F32R, tag="wo")
                            nc.sync.dma_start(
                                wo[:],
                                WoT_d[:, 512 * e:512 * e + 512].rearrange(
                                    "(cc p) d -> p cc d", p=128
                                ),
                            )
                            for tsub in range(2 * NBO):
                                fps = psf.tile([128, 512], F32, tag="fps")
                                for d in range(CC):
                                    nc.tensor.matmul(
                                        fps[:],
                                        otps[tsub][:, d, :],
                                        wo[:, d, :],
                                        start=(d == 0),
                                        stop=(d == CC - 1),
                                    )
                                yt = stage.tile([128, 512], F32, tag="yt")
                                nc.vector.tensor_scalar_mul(
                                    yt[:], fps[:], recipt[:, tsub:tsub + 1]
                                )
                                nc.sync.dma_start(
                                    y_d[128 * tsub:128 * tsub + 128, 512 * e:512 * e + 512],
                                    yt[:],
                                )
                    pool_cst_cm.__exit__(None, None, None)
    nc.compile()
    return nc


def _host_prep(x, Wq, Wk, Wv, Wo, T_, C_):
    NBO = T_ // 512
    x = np.asarray(x, np.float32)
    WqT = np.ascontiguousarray(np.asarray(Wq, np.float32).T)
    WkT = np.ascontiguousarray(np.asarray(Wk, np.float32).T)
    WvT = np.ascontiguousarray(np.asarray(Wv, np.float32).T)
    WoT = np.ascontiguousarray(np.asarray(Wo, np.float32).T)
    ones = np.ones((128, 2), np.float32)
    masks = {}
    own_cols = {}
    for h in range(2):
        mb = np.zeros((NBO, 4, 128, 256), np.float32)
        for p in range(NBO):
            g = 2 * p + h
            t0 = 256 * g
            for kl in range(4):
                s0 = 512 * p + 128 * kl
                s_idx = s0 + np.arange(128)[:, None]
                t_idx = t0 + np.arange(256)[None, :]
                mb[p, kl] = np.where(s_idx <= t_idx, 0.0, NEG)
        masks[h] = mb
        own_cols[h] = np.concatenate(
            [np.arange(256 * (2 * p + h), 256 * (2 * p + h) + 256) for p in range(NBO)]
        )
    in_maps = []
    for core in range(8):
        b, h = core // 2, core % 2
        xb = x[b % x.shape[0]]
        xT = np.ascontiguousarray(xb.T)
        xTq = np.ascontiguousarray(xT[:, own_cols[h]])
        xTh = np.ascontiguousarray(xT[:, h * (xT.shape[1] // 2):(h + 1) * (xT.shape[1] // 2)])
        in_maps.append(
            {
                "xTh": xTh,
                "xTq": xTq,
                "WqT": WqT,
                "WkT": WkT,
                "WvT": WvT,
                "WoT": WoT,
                "mb": masks[h],
                "ones": ones,
            }
        )
    return in_maps, own_cols


def kernel(x, Wq, Wk, Wv, Wo):
    from concourse.bass_utils import run_bass_kernel_spmd

    T_, C_ = T_FULL, C_FULL
    key = (T_, C_)
    if key not in _CACHE:
        _CACHE[key] = _build(T_, C_)
    nc = _CACHE[key]
    in_maps, own_cols = _host_prep(x, Wq, Wk, Wv, Wo, T_, C_)
    res = run_bass_kernel_spmd(nc, in_maps, list(range(8)))
    NBO = T_ // 512
    y = np.zeros((B, T_, C_), np.float32)
    for core in range(8):
        b, h = core // 2, core % 2
        yc = res.results[core]["y"]
        for p in range(NBO):
            g = 2 * p + h
            y[b, 256 * g:256 * g + 256, :] = yc[256 * p:256 * p + 256, :]
    return y



# revision 2
# speedup vs baseline: 2.6620x; 2.6620x over previous
"""Trainium2 Bass kernel for single-head causal attention (B=4, T=2048, C=2048).

Sharding: 8 cores = 4 batches x 2 t-interleave. Core (b, h) owns the 256-row
blocks {h, 2+h, 4+h, 6+h} of batch b (interleaved for causal load balance).
The two cores of a batch each compute HALF of K.T and V and exchange via a
2-core AllGather (overlapped with the Q projection). Attention runs in the
"transposed domain" (scores.T = [s, t]) so every matmul consumes naturally
laid-out operands: exp(scale*s + additive mask) without normalization, softmax
denominators via ones-matmul partition reduction, folded in as a per-partition
scale on the final-projection output, which lands in natural [t, e] layout.
Host pre-transposes x / weights (part of sharding prep) and gathers per-core
outputs.

All matmuls run in bf16 (fp32 PSUM accumulation). Unlike fp32r -- whose
matmuls self-load the 128-row stationary operand every instruction (a
(128+N)-cycle cost) -- bf16 weights load via separate LDWEIGHTS with fast
weight load, double-buffered behind the previous matmul's streaming, so a
matmul costs ~N cycles. bf16 also halves DMA + collective bytes. K.T, V and
Q.T stay resident in SBUF for the whole attention phase (no DRAM round-trips
inside the k-loops).
"""
import sys

sys.path.insert(0, "/opt/trn_rl_repo")
import numpy as np
from ml_dtypes import bfloat16

_CACHE = {}

B = 4
T_FULL = 2048
C_FULL = 2048
NEG = -1e30


def _build(T_, C_, reps=1):
    import concourse.bacc as bacc
    import concourse.mybir as mybir
    import concourse.tile as tile

    F32 = mybir.dt.float32
    BF16 = mybir.dt.bfloat16
    AF = mybir.ActivationFunctionType
    SCALE = 1.0 / float(np.sqrt(C_FULL))

    CC = C_ // 128      # contraction 128-chunks (also d-chunks)
    NE = C_ // 512      # e-512 chunks for the final projection
    NBO = T_ // 512     # owned 256-blocks per core (j range)
    TOWN = NBO * 256    # owned rows per core
    SK = T_ // 512      # s-512 chunks (KT scratch blocks)
    NS = T_ // 128      # s-128 chunks
    NQ = TOWN // 512    # q 512-col chunks

    nc = bacc.Bacc("TRN2", target_bir_lowering=False, debug=False, num_devices=8)
    xTh_d = nc.declare_dram_parameter("xTh", [C_, T_ // 2], BF16, isOutput=False)
    xTq_d = nc.declare_dram_parameter("xTq", [C_, TOWN], BF16, isOutput=False)
    WqT_d = nc.declare_dram_parameter("WqT", [C_, C_], BF16, isOutput=False)
    WkT_d = nc.declare_dram_parameter("WkT", [C_, C_], BF16, isOutput=False)
    WvT_d = nc.declare_dram_parameter("WvT", [C_, C_], BF16, isOutput=False)
    WoT_d = nc.declare_dram_parameter("WoT", [C_, C_], BF16, isOutput=False)
    mb_d = nc.declare_dram_parameter("mb", [NBO, 4, 128, 256], F32, isOutput=False)
    ones_d = nc.declare_dram_parameter("ones", [128, 2], BF16, isOutput=False)
    y_d = nc.declare_dram_parameter("y", [TOWN, C_], F32, isOutput=True)

    with tile.TileContext(nc) as tc:
        with tc.tile_pool(name="dram", bufs=1, space="DRAM") as dram:
            KT_s = dram.tile([SK, C_, 512], BF16, tag="kts")
            V_s = dram.tile([T_, C_], BF16, tag="vs")
            KT_h = dram.tile([SK // 2, C_, 512], BF16, tag="kth")
            V_h = dram.tile([T_ // 2, C_], BF16, tag="vh")
            OT_s = dram.tile([C_, TOWN], BF16, tag="ots")

            with tc.tile_pool(name="stage", bufs=4) as stage:
                for _rep in range(reps):
                    # ======== P1a: K.T = WkT.T @ xT  -> KT_s [d, s] ========
                    with tc.tile_pool(name="xt", bufs=1) as pool_xt:
                        xt = pool_xt.tile([128, CC, T_ // 2], BF16, tag="xt")
                        for c in range(CC):
                            nc.sync.dma_start(xt[:, c, :], xTh_d[128 * c:128 * c + 128, :])
                        with (
                            tc.tile_pool(name="wk", bufs=2) as pool_w,
                            tc.tile_pool(name="psk", bufs=8, space="PSUM") as psk,
                        ):
                            for d in range(CC):
                                wk = pool_w.tile([128, CC, 128], BF16, tag="wk")
                                nc.sync.dma_start(
                                    wk[:],
                                    WkT_d[:, 128 * d:128 * d + 128].rearrange(
                                        "(cc p) d -> p cc d", p=128
                                    ),
                                )
                                kps = [psk.tile([128, 512], F32, tag="kps", name=f"kps{d}_{ss}") for ss in range(SK // 2)]
                                for c in range(CC):
                                    for ss in range(SK // 2):
                                        nc.tensor.matmul(
                                            kps[ss][:],
                                            wk[:, c, :],
                                            xt[:, c, 512 * ss:512 * ss + 512],
                                            start=(c == 0),
                                            stop=(c == CC - 1),
                                        )
                                for ss in range(SK // 2):
                                    st = stage.tile([128, 512], BF16, tag="st512")
                                    nc.vector.tensor_copy(st[:], kps[ss][:])
                                    nc.sync.dma_start(
                                        KT_h[ss, 128 * d:128 * d + 128, :], st[:]
                                    )
                        # ======== P1b: V = xT.T @ WvT -> V_s [s, d] ========
                        with (
                            tc.tile_pool(name="wv", bufs=2) as pool_wv,
                            tc.tile_pool(name="psv", bufs=4, space="PSUM") as psv,
                        ):
                            for dd in range(C_ // 512):
                                wv = pool_wv.tile([128, CC, 512], BF16, tag="wv")
                                nc.sync.dma_start(
                                    wv[:],
                                    WvT_d[:, 512 * dd:512 * dd + 512].rearrange(
                                        "(cc p) d -> p cc d", p=128
                                    ),
                                )
                                for s in range(NS // 2):
                                    vps = psv.tile([128, 512], F32, tag="vps")
                                    for c in range(CC):
                                        nc.tensor.matmul(
                                            vps[:],
                                            xt[:, c, 128 * s:128 * s + 128],
                                            wv[:, c, :],
                                            start=(c == 0),
                                            stop=(c == CC - 1),
                                        )
                                    st = stage.tile([128, 512], BF16, tag="st512")
                                    nc.vector.tensor_copy(st[:], vps[:])
                                    nc.sync.dma_start(
                                        V_h[128 * s:128 * s + 128, 512 * dd:512 * dd + 512],
                                        st[:],
                                    )
                    # ======== exchange K/V halves within batch pairs ========
                    groups = [[0, 1], [2, 3], [4, 5], [6, 7]]
                    nc.gpsimd.collective_compute(
                        "AllGather",
                        mybir.AluOpType.bypass,
                        replica_groups=groups,
                        ins=[KT_h[:]],
                        outs=[KT_s[:]],
                    )
                    nc.gpsimd.collective_compute(
                        "AllGather",
                        mybir.AluOpType.bypass,
                        replica_groups=groups,
                        ins=[V_h[:]],
                        outs=[V_s[:]],
                    )
                    with tc.tile_pool(name="qsb", bufs=1) as pool_qsb:
                        # ======== P1c: Q.T = WqT.T @ xTq -> QT_sb (SBUF-resident) ========
                        QT_sb = pool_qsb.tile([128, CC, TOWN], BF16, tag="qtsb")
                        with (
                            tc.tile_pool(name="xtq", bufs=1) as pool_xtq,
                            tc.tile_pool(name="wq", bufs=2) as pool_wq,
                            tc.tile_pool(name="psq", bufs=4, space="PSUM") as psq,
                        ):
                            xtq = pool_xtq.tile([128, CC, TOWN], BF16, tag="xtq")
                            for c in range(CC):
                                nc.sync.dma_start(xtq[:, c, :], xTq_d[128 * c:128 * c + 128, :])
                            for d in range(CC):
                                wq = pool_wq.tile([128, CC, 128], BF16, tag="wq")
                                nc.sync.dma_start(
                                    wq[:],
                                    WqT_d[:, 128 * d:128 * d + 128].rearrange(
                                        "(cc p) d -> p cc d", p=128
                                    ),
                                )
                                for tt in range(NQ):
                                    qps = psq.tile([128, 512], F32, tag="qps")
                                    for c in range(CC):
                                        nc.tensor.matmul(
                                            qps[:],
                                            wq[:, c, :],
                                            xtq[:, c, 512 * tt:512 * tt + 512],
                                            start=(c == 0),
                                            stop=(c == CC - 1),
                                        )
                                    nc.vector.tensor_copy(
                                        QT_sb[:, d, 512 * tt:512 * tt + 512], qps[:]
                                    )

                        # ======== P2: attention per owned block j ========
                        pool_cst_cm = tc.tile_pool(name="cst", bufs=1)
                        pool_cst = pool_cst_cm.__enter__()
                        onest = pool_cst.tile([128, 2], BF16, tag="ones")
                        nc.sync.dma_start(onest[:], ones_d[:])
                        recipt = pool_cst.tile([128, 2 * NBO], F32, tag="recip")
                        with (
                            tc.tile_pool(name="kvsb", bufs=1) as pool_kvsb,
                            tc.tile_pool(name="mbp", bufs=1) as pool_mb,
                            tc.tile_pool(name="attn", bufs=8 * NBO - 4) as pool_attn,
                        ):
                            # K.T and V resident in SBUF for all of P2
                            KT_sb = pool_kvsb.tile([128, CC, T_], BF16, tag="ktsb")
                            for kk in range(SK):
                                nc.sync.dma_start(
                                    KT_sb[:, :, 512 * kk:512 * kk + 512],
                                    KT_s[kk].rearrange("(cc p) s -> p cc s", p=128),
                                )
                            V_sb = pool_kvsb.tile([128, NS, C_], BF16, tag="vsb")
                            nc.sync.dma_start(
                                V_sb[:], V_s[:].rearrange("(ns p) d -> p ns d", p=128)
                            )
                            mbt = pool_mb.tile([128, NBO, 4, 256], F32, tag="mb")
                            nc.sync.dma_start(mbt[:], mb_d[:].rearrange("nb k p n -> p nb k n"))

                            # owned blocks processed in PAIRS (one K-chunk
                            # stationary load serves two scores matmuls)
                            for grp in range(NBO // 2):
                                js = [2 * grp, 2 * grp + 1]
                                attn = {}
                                for j in js:
                                    attn[j] = [
                                        pool_attn.tile(
                                            [128, 256], BF16, tag="attn", name=f"attn{j}_{k}"
                                        )
                                        for k in range(4 * j + 4)
                                    ]
                                with tc.tile_pool(name="pssc", bufs=4, space="PSUM") as pssc:
                                    for kk in range(2 * grp + 2):
                                        for kl in range(4):
                                            k = 4 * kk + kl
                                            jlist = [j for j in js if 4 * j + 3 >= k]
                                            sps = {
                                                j: pssc.tile(
                                                    [128, 256], F32, tag="sps",
                                                    name=f"sps{grp}_{k}_{j}",
                                                )
                                                for j in jlist
                                            }
                                            for d in range(CC):
                                                for j in jlist:
                                                    nc.tensor.matmul(
                                                        sps[j][:],
                                                        KT_sb[:, d, 128 * k:128 * k + 128],
                                                        QT_sb[:, d, 256 * j:256 * j + 256],
                                                        start=(d == 0),
                                                        stop=(d == CC - 1),
                                                    )
                                            for j in jlist:
                                                if k >= 4 * j:
                                                    nc.vector.tensor_add(
                                                        sps[j][:], sps[j][:],
                                                        mbt[:, j, k - 4 * j, :],
                                                    )
                                                nc.scalar.activation(
                                                    attn[j][k][:], sps[j][:], AF.Exp,
                                                    scale=SCALE,
                                                )
                                with tc.tile_pool(name="psr", bufs=2, space="PSUM") as psr:
                                    for j in js:
                                        for sub in range(2):
                                            rps = psr.tile([128, 2], F32, tag="rps")
                                            for k in range(4 * j + 4):
                                                nc.tensor.matmul(
                                                    rps[:],
                                                    attn[j][k][:, 128 * sub:128 * sub + 128],
                                                    onest[:],
                                                    start=(k == 0),
                                                    stop=(k == 4 * j + 3),
                                                )
                                            nc.vector.reciprocal(
                                                recipt[:, 2 * j + sub:2 * j + sub + 1],
                                                rps[:, 0:1],
                                            )
                                for j in js:
                                    n_k = 4 * j + 4
                                    with tc.tile_pool(
                                        name="psav", bufs=CC // 2, space="PSUM"
                                    ) as psav:
                                        avs = [
                                            psav.tile(
                                                [128, 512], F32, tag="av", name=f"av{j}_{dp}"
                                            )
                                            for dp in range(CC // 2)
                                        ]
                                        for dp in range(CC // 2):
                                            nc.vector.memset(avs[dp][:], 0.0)
                                        for k in range(n_k):
                                            for d in range(CC):
                                                nc.tensor.matmul(
                                                    avs[d // 2][:, 256 * (d % 2):256 * (d % 2) + 256],
                                                    V_sb[:, k, 128 * d:128 * d + 128],
                                                    attn[j][k][:],
                                                    start=False,
                                                    stop=(k == n_k - 1),
                                                    skip_group_check=True,
                                                )
                                        for d in range(CC):
                                            st = stage.tile([128, 256], BF16, tag="st256")
                                            nc.vector.tensor_copy(
                                                st[:],
                                                avs[d // 2][:, 256 * (d % 2):256 * (d % 2) + 256],
                                            )
                                            nc.sync.dma_start(
                                                OT_s[128 * d:128 * d + 128, 256 * j:256 * j + 256],
                                                st[:],
                                            )

                        # ======== P3: y = (OT.T @ WoT) * recip ========
                        with (
                            tc.tile_pool(name="wo", bufs=2) as pool_wo,
                            tc.tile_pool(name="ot", bufs=2 * NBO) as pool_ot,
                            tc.tile_pool(name="psf", bufs=4, space="PSUM") as psf,
                        ):
                            otps = []
                            for tsub in range(2 * NBO):
                                otp = pool_ot.tile([128, CC, 128], BF16, tag="ot", name=f"otp{tsub}")
                                nc.sync.dma_start(
                                    otp[:],
                                    OT_s[:, 128 * tsub:128 * tsub + 128].rearrange(
                                        "(cc p) t -> p cc t", p=128
                                    ),
                                )
                                otps.append(otp)
                            for e in range(NE):
                                wo = pool_wo.tile([128, CC, 512], BF16, tag="wo")
                                nc.sync.dma_start(
                                    wo[:],
                                    WoT_d[:, 512 * e:512 * e + 512].rearrange(
                                        "(cc p) d -> p cc d", p=128
                                    ),
                                )
                                for tsub in range(2 * NBO):
                                    fps = psf.tile([128, 512], F32, tag="fps")
                                    for d in range(CC):
                                        nc.tensor.matmul(
                                            fps[:],
                                            otps[tsub][:, d, :],
                                            wo[:, d, :],
                                            start=(d == 0),
                                            stop=(d == CC - 1),
                                        )
                                    yt = stage.tile([128, 512], F32, tag="yt")
                                    nc.vector.tensor_scalar_mul(
                                        yt[:], fps[:], recipt[:, tsub:tsub + 1]
                                    )
                                    nc.sync.dma_start(
                                        y_d[128 * tsub:128 * tsub + 128, 512 * e:512 * e + 512],
                                        yt[:],
                                    )
                        pool_cst_cm.__exit__(None, None, None)
    nc.compile()
    return nc


def _host_prep(x, Wq, Wk, Wv, Wo, T_, C_):
    NBO = T_ // 512
    x = np.asarray(x, np.float32)
    WqT = np.ascontiguousarray(np.asarray(Wq, np.float32).T).astype(bfloat16)
    WkT = np.ascontiguousarray(np.asarray(Wk, np.float32).T).astype(bfloat16)
    WvT = np.ascontiguousarray(np.asarray(Wv, np.float32).T).astype(bfloat16)
    WoT = np.ascontiguousarray(np.asarray(Wo, np.float32).T).astype(bfloat16)
    ones = np.ones((128, 2), bfloat16)
    masks = {}
    own_cols = {}
    for h in range(2):
        mb = np.zeros((NBO, 4, 128, 256), np.float32)
        for p in range(NBO):
            g = 2 * p + h
            t0 = 256 * g
            for kl in range(4):
                s0 = 512 * p + 128 * kl
                s_idx = s0 + np.arange(128)[:, None]
                t_idx = t0 + np.arange(256)[None, :]
                mb[p, kl] = np.where(s_idx <= t_idx, 0.0, NEG)
        masks[h] = mb
        own_cols[h] = np.concatenate(
            [np.arange(256 * (2 * p + h), 256 * (2 * p + h) + 256) for p in range(NBO)]
        )
    in_maps = []
    for core in range(8):
        b, h = core // 2, core % 2
        xb = x[b % x.shape[0]]
        xT = np.ascontiguousarray(xb.T).astype(bfloat16)
        xTq = np.ascontiguousarray(xT[:, own_cols[h]])
        xTh = np.ascontiguousarray(xT[:, h * (xT.shape[1] // 2):(h + 1) * (xT.shape[1] // 2)])
        in_maps.append(
            {
                "xTh": xTh,
                "xTq": xTq,
                "WqT": WqT,
                "WkT": WkT,
                "WvT": WvT,
                "WoT": WoT,
                "mb": masks[h],
                "ones": ones,
            }
        )
    return in_maps, own_cols


def kernel(x, Wq, Wk, Wv, Wo):
    from concourse.bass_utils import run_bass_kernel_spmd

    T_, C_ = T_FULL, C_FULL
    key = (T_, C_)
    if key not in _CACHE:
        _CACHE[key] = _build(T_, C_)
    nc = _CACHE[key]
    in_maps, own_cols = _host_prep(x, Wq, Wk, Wv, Wo, T_, C_)
    res = run_bass_kernel_spmd(nc, in_maps, list(range(8)))
    NBO = T_ // 512
    y = np.zeros((B, T_, C_), np.float32)
    for core in range(8):
        b, h = core // 2, core % 2
        yc = res.results[core]["y"]
        for p in range(NBO):
            g = 2 * p + h
            y[b, 256 * g:256 * g + 256, :] = yc[256 * p:256 * p + 256, :]
    return y
